# revision 29
# baseline (speedup 1.0000x reference)
"""Trainium2 Bass kernel for the AUV Fossen dynamics RK2 step (nn_AUVFossen).

Per row (batch K): x[13] = pos(3)+quat(x,y,z,w)(4)+v(6), u[6].
  k1 = f(x,u); k2 = f(x + DT*k1, u); out = normquat(x + DT/2*(k1+k2))

Sharding: pure data parallel over 8 NeuronCores (batch split).
Layout: batch-major [128, C*W] tiles (W rows/partition, comps interleaved).
Compute is planar elementwise split across DVE + ACT:
  DVE : fp16 quat/rotation/S/cross subgraph (tensor_tensor runs in 2x mode
        for 2-byte dtypes) + fp32 RHS accumulation (1x)
  ACT : planarize (strided->planar, dtype-converting), |qd_i*v_i|, squares,
        sqrt, deplanarize of the bf16 planar output block
GPSIMD is deliberately unused: its tensor ops run at ~7 cyc/elem here and
contend with the DVE for the shared SBUF port.

Precision: the RHS accumulation is fp32 (dot1's RHS error is amplified
~0.2x-abs into the output via the quadratic damping of dot 2); the fp16
subgraph keeps monomial errors ~8x below bf16, which measurement showed
is needed. The va x vl cross products of dot 2 can exceed fp16 range
(|v2|^2 up to ~2.5e5), so those run in bf16.
"""

import os
import sys

for _p in ("/opt/trn_rl_repo", "/root/.axon_site/_ro/trn_rl_repo"):
    if os.path.isdir(_p) and _p not in sys.path:
        sys.path.insert(0, _p)

import numpy as np

import concourse.bacc as bacc
import concourse.bass as bass
import concourse.mybir as mybir
from concourse.alu_op_type import AluOpType
from concourse.tile import TileContext

F32 = mybir.dt.float32
F16 = mybir.dt.float16
BF16 = mybir.dt.bfloat16
AF = mybir.ActivationFunctionType
MULT = AluOpType.mult
ADD = AluOpType.add
SUB = AluOpType.subtract

DT = 0.1
GRAVITY = 9.81
DENSITY = 1028.0

N_CORES = 8
P = 128
NSF = 50          # fp32 scratch slots
NSB = 64          # fp16 scratch slots
NSC = 24          # bf16 scratch slots (cross products + planar output)

TRACE = False          # set by test.py
LAST_RUN_INFO = {}


class CompView:
    """Component view of a [128, C*W] interleaved tile (addr = w*C + c)."""

    def __init__(self, tile_ap, wstride, base, W):
        self.t = tile_ap.tensor
        self.off = tile_ap.offset + base
        self.part = tile_ap.ap[0]
        self.ws = wstride
        self.W = W

    def ap(self, c0, n=1, cstep=1, w0=0, wn=None):
        wn = self.W if wn is None else wn
        off = self.off + c0 + w0 * self.ws
        if n == 1:
            return bass.AP(self.t, off, [self.part, [self.ws, wn]])
        return bass.AP(self.t, off,
                       [self.part, [cstep, n], [self.ws, wn]])

    def bcast(self, c0, n):
        return bass.AP(self.t, self.off + c0,
                       [self.part, [0, n], [self.ws, self.W]])


class Region:
    """Contiguous run of planar scratch slots (slot = [128, W] plane)."""

    def __init__(self, scr, slot0, n):
        self.scr = scr
        self.slot0 = slot0
        self.n = n

    def ap(self, s0=0, n=1, sstep=1, w0=0, wn=None):
        scr = self.scr
        wn = scr.W if wn is None else wn
        off = scr.off + (self.slot0 + s0) * scr.W + w0
        if n == 1:
            return bass.AP(scr.t, off, [scr.part, [1, wn]])
        return bass.AP(scr.t, off,
                       [scr.part, [sstep * scr.W, n], [1, wn]])

    def bcast(self, s0, n):
        scr = self.scr
        off = scr.off + (self.slot0 + s0) * scr.W
        return bass.AP(scr.t, off, [scr.part, [0, n], [1, scr.W]])


class Scratch:
    def __init__(self, tile_ap, nslots, W):
        self.t = tile_ap.tensor
        self.off = tile_ap.offset
        self.part = tile_ap.ap[0]
        self.W = W
        self.free_slots = set(range(nslots))
        self.regions = {}

    def alloc(self, name, n, high=False):
        fs = sorted(self.free_slots)
        run = None
        idxs = range(len(fs) - n + 1)
        if high:
            idxs = reversed(list(idxs))
        for i in idxs:
            if fs[i + n - 1] - fs[i] == n - 1:
                run = fs[i]
                break
        assert run is not None, f"scratch OOM for {name}({n}); free={len(fs)}"
        for s in range(run, run + n):
            self.free_slots.remove(s)
        self.regions[name] = (run, n)
        return Region(self, run, n)

    def free(self, *names):
        for name in names:
            run, n = self.regions.pop(name)
            self.free_slots.update(range(run, run + n))


def _extract_params(inputs):
    mass = float(np.asarray(inputs["mass"]).reshape(-1)[0])
    volume = float(np.asarray(inputs["volume"]).reshape(-1)[0])
    cog = np.asarray(inputs["cog"], np.float64).reshape(3)
    cob = np.asarray(inputs["cob"], np.float64).reshape(3)
    mTot = np.asarray(inputs["mTot"], np.float64).reshape(6, 6)
    linDamp = np.asarray(inputs["linDamp"], np.float64).reshape(6, 6)
    linDampFow = np.asarray(inputs["linDampFow"], np.float64).reshape(6, 6)
    quadDamp = np.asarray(inputs["quadDamp"], np.float64).reshape(6, 6)

    scale = max(np.abs(mTot).max(), 1e-30)
    tl, tr = mTot[0:3, 0:3], mTot[0:3, 3:6]
    bl, br = mTot[3:6, 0:3], mTot[3:6, 3:6]
    m1 = float(np.trace(tl) / 3.0)
    m2 = float(np.trace(br) / 3.0)
    structured = (
        np.abs(tl - m1 * np.eye(3)).max() < 1e-5 * scale
        and np.abs(br - m2 * np.eye(3)).max() < 1e-5 * scale
        and np.abs(tr).max() < 1e-5 * scale
        and np.abs(bl).max() < 1e-5 * scale
    )
    if not structured:
        raise NotImplementedError("unstructured mTot not supported")
    if np.abs(linDampFow).max() > 1e-30:
        raise NotImplementedError("nonzero linDampFow not supported")

    minv = np.diag(np.linalg.inv(mTot))
    ld = linDamp.copy()                 # -Dv_lin = +linDamp @ v
    qd = np.diag(quadDamp)              # only diag of quadDamp matters
    c1 = GRAVITY * (volume * DENSITY - mass)
    c2 = -mass * GRAVITY * cog + volume * DENSITY * GRAVITY * cob
    return dict(m1=m1, minv=minv, ld=ld, qd=qd, c1=float(c1), c2=c2)


def _runs(idxs):
    out = []
    for i in idxs:
        if out and i == out[-1][0] + out[-1][1]:
            out[-1] = (out[-1][0], out[-1][1] + 1)
        else:
            out.append((i, 1))
    return out


def _val_runs(vals, nonzero_only=False):
    """Group consecutive equal values: [(i0, n, val)]."""
    out = []
    for i, v in enumerate(vals):
        if nonzero_only and v == 0.0:
            continue
        if out and out[-1][2] == v and i == out[-1][0] + out[-1][1]:
            out[-1] = (out[-1][0], out[-1][1] + 1, v)
        else:
            out.append((i, 1, v))
    return out


def build_program(pp, K_core, W):
    assert K_core % (P * W) == 0
    n_chunks = K_core // (P * W)

    nc = bacc.Bacc("TRN2", target_bir_lowering=False, debug=False,
                   num_devices=N_CORES)
    x_d = nc.dram_tensor("x", (K_core, 13), F32, kind="ExternalInput")
    u_d = nc.dram_tensor("u", (K_core, 6), F32, kind="ExternalInput")
    o_d = nc.dram_tensor("o", (K_core, 13), F32, kind="ExternalOutput")

    xr = x_d.rearrange("(n p w) c -> n p (w c)", p=P, w=W)
    ur = u_d.rearrange("(n p w) c -> n p (w c)", p=P, w=W)
    orr = o_d.rearrange("(n p w) c -> n p (w c)", p=P, w=W)

    nb = 2 if n_chunks > 1 else 1
    with TileContext(nc) as tc:
        with tc.tile_pool(name="io", bufs=nb) as iop, \
             tc.tile_pool(name="pv", bufs=nb) as pvp, \
             tc.tile_pool(name="scr", bufs=1) as scrp:
            assert n_chunks <= 2, "tile pre-creation assumes <=2 chunks"
            tiles = []
            for ci in range(n_chunks):
                X = iop.tile([P, 13 * W], F32, tag="X")
                U = iop.tile([P, 6 * W], F32, tag="U")
                O = iop.tile([P, 13 * W], F32, tag="O")
                XB = pvp.tile([P, 13 * W], BF16, tag="XB")
                XF = pvp.tile([P, 10 * W], F32, tag="XF")
                UF = pvp.tile([P, 6 * W], BF16, tag="UF")
                SCF = scrp.tile([P, NSF * W], F32, tag="SCF")
                SCB = scrp.tile([P, NSB * W], BF16, tag="SCB")
                SCC = scrp.tile([P, NSC * W], BF16, tag="SCC")
                tiles.append((X, U, O, XB, XF, UF, SCF, SCB, SCC))

            def in_dmas(ci, eng=None):
                eng = eng or nc.sync
                # X lands in four w-quarters: each dma_start only spreads
                # over a few DMA queues, so more smaller transfers engage
                # more queues in parallel and the planarize (and with it
                # the DVE) starts sooner. Later chunks' DMAs are issued
                # after earlier ones so they do not steal queue bandwidth
                # from the chunk that gates the DVE.
                X, U = tiles[ci][0], tiles[ci][1]
                h = 13 * (W // 2)
                eng.dma_start(X[:, :h], xr[ci][:, :h])
                eng.dma_start(X[:, h:], xr[ci][:, h:])
                eng.dma_start(U[:, :], ur[ci])

            in_dmas(0)
            emit_planarize(nc, tiles[0][0][:, :], tiles[0][1][:, :],
                           tiles[0][3][:, :], tiles[0][4][:, :],
                           tiles[0][5][:, :], W)
            for ci in range(n_chunks):
                X, U, O, XB, XF, UF, SCF, SCB, SCC = tiles[ci]
                last = ci == n_chunks - 1
                plan_next = None
                if not last:
                    nxt = tiles[ci + 1]
                    plan_next = (lambda nx=nxt: emit_planarize(
                        nc, nx[0][:, :], nx[1][:, :], nx[3][:, :],
                        nx[4][:, :], nx[5][:, :], W))
                after_dot1 = None
                if not last:
                    after_dot1 = (lambda cj=ci + 1: in_dmas(cj, nc.scalar))
                tail_dmas = []
                emit_chunk(nc, pp, X[:, :], U[:, :], O[:, :],
                           XB[:, :], XF[:, :], UF[:, :], SCF[:, :],
                           SCB[:, :], SCC[:, :], W,
                           split_tail=last, tail_dmas=tail_dmas,
                           plan_next=plan_next, after_dot1=after_dot1)
                if last and tail_dmas:
                    for w0, wn in tail_dmas:
                        nc.sync.dma_start(orr[ci][:, 13 * w0:13 * (w0 + wn)],
                                          O[:, 13 * w0:13 * (w0 + wn)])
                else:
                    nc.sync.dma_start(orr[ci], O[:, :])
    nc.compile()
    return nc


def emit_planarize(nc, Xt, Ut, XBt, XFt, UFt, W):
    # planarize on ACT: XB bf16 = [q(0:4) vl(4:7) va(7:10) pos(10:13)],
    # XF fp32 = [q(0:4) v(4:10)] (feeds dot1's error-amplified RHS path),
    # UF bf16 = u. Emitted in w-halves so each copy can start as soon as
    # its half of X has landed; q/v copies go first (they gate the DVE).
    a = nc.scalar
    xall = CompView(Xt, 13, 0, W)
    uall = CompView(Ut, 6, 0, W)
    XBR = Region(Scratch(XBt, 13, W), 0, 13)
    XFR = Region(Scratch(XFt, 10, W), 0, 10)
    UFR = Region(Scratch(UFt, 6, W), 0, 6)
    hw = W // 2
    for w0, wn in ((0, hw), (hw, W - hw)):
        a.activation(XBR.ap(0, 10, w0=w0, wn=wn),
                     xall.ap(3, 10, w0=w0, wn=wn), AF.Copy)
    for w0, wn in ((0, hw), (hw, W - hw)):
        a.activation(XFR.ap(0, 10, w0=w0, wn=wn),
                     xall.ap(3, 10, w0=w0, wn=wn), AF.Copy)


def emit_chunk(nc, pp, Xt, Ut, Ot, XBt, XFt, UFt, SCFt, SCBt, SCCt, W,
               split_tail=False, tail_dmas=None, plan_next=None,
               after_dot1=None):
    v = nc.vector
    a = nc.scalar
    scrf = Scratch(SCFt, NSF, W)
    scrb = Scratch(SCBt, NSB, W)
    scrc = Scratch(SCCt, NSC, W)
    xb = Scratch(XBt, 13, W)
    xf = Scratch(XFt, 10, W)
    uf = Scratch(UFt, 6, W)

    oint = CompView(Ot, 13, 0, W)

    xq = Region(xb, 0, 4)
    vl1 = Region(xb, 4, 3)
    va1 = Region(xb, 7, 3)
    xpos = Region(xb, 10, 3)
    qF = Region(xf, 0, 4)
    vF1 = Region(xf, 4, 6)
    UF = Region(uf, 0, 6)

    OPL = scrc.alloc("OPL", 13)   # [pos(0:3) quat(3:7) v(7:13)] bf16

    S1R = scrb.alloc("S1R", 4)
    S2R = scrb.alloc("S2R", 4)
    PL1 = scrb.alloc("PL1", 3)
    PL2 = scrb.alloc("PL2", 3)
    Q2 = scrb.alloc("Q2", 4)
    V2B = scrb.alloc("V2B", 6)
    RHS1 = scrf.alloc("RHS1", 6)
    RHS2 = scrf.alloc("RHS2", 6)
    V2F = scrf.alloc("V2F", 6)

    emit_dot(nc, pp, scrf, scrb, scrc, xq, vl1, va1, vF1, UF,
             S1R, RHS1, PL1, prec=True, qf=qF)

    # planarize pos/u late: needed only by the RHS assembly and outputs,
    # and ahead of dot1's ACT feeders they delay the DVE's monomials.
    xall_c = CompView(Xt, 13, 0, W)
    uall_c = CompView(Ut, 6, 0, W)
    a.activation(xpos.ap(0, 3), xall_c.ap(0, 3), AF.Copy)
    a.activation(UF.ap(0, 6), uall_c.ap(0, 6), AF.Copy)
    if after_dot1 is not None:
        after_dot1()

    # x2 = x + DT*k1 (quat bf16; v fp32 + bf16 copy for dot2's products)
    v.scalar_tensor_tensor(Q2.ap(0), S1R.ap(0), -DT / 2, xq.ap(0), MULT, ADD)
    v.scalar_tensor_tensor(Q2.ap(1, 3), S1R.ap(1, 3), DT / 2, xq.ap(1, 3),
                           MULT, ADD)
    mv = pp["minv"]
    for i0, n, mval in _val_runs(mv):
        v.scalar_tensor_tensor(V2F.ap(i0, n), RHS1.ap(i0, n), DT * mval,
                               vF1.ap(i0, n), MULT, ADD)
    a.activation(V2B.ap(0, 6), V2F.ap(0, 6), AF.Copy)

    norm_regs = {}

    def _norm_chain():
        SS = scrb.alloc("SS", 4, high=True)
        QR = scrb.alloc("QR", 4, high=True)
        v.tensor_tensor(SS.ap(0, 4), S1R.ap(0, 4), S2R.ap(0, 4), ADD)
        v.scalar_tensor_tensor(QR.ap(0), SS.ap(0), -DT / 4, xq.ap(0), MULT, ADD)
        v.scalar_tensor_tensor(QR.ap(1, 3), SS.ap(1, 3), DT / 4, xq.ap(1, 3),
                               MULT, ADD)
        scrb.free("SS")
        NQ = scrf.alloc("NQ", 4, high=True)
        NS2 = scrf.alloc("NS2", 2, high=True)
        NS1 = scrf.alloc("NS1", 1, high=True)
        SQC = scrf.alloc("SQC", 1, high=True)
        RINV = scrf.alloc("RINV", 1, high=True)
        a.activation(NQ.ap(0, 4), QR.ap(0, 4), AF.Square)
        v.tensor_tensor(NS2.ap(0, 2), NQ.ap(0, 2), NQ.ap(2, 2), ADD)
        v.tensor_tensor(NS1.ap(0), NS2.ap(0), NS2.ap(1), ADD)
        a.activation(SQC.ap(0), NS1.ap(0), AF.Sqrt)
        v.reciprocal_approx_fast(RINV.ap(0), SQC.ap(0))
        scrf.free("NQ", "NS2", "NS1", "SQC")
        norm_regs["QR"] = QR
        norm_regs["RINV"] = RINV

    emit_dot(nc, pp, scrf, scrb, scrc, Q2, Region(scrb, V2B.slot0, 3),
             Region(scrb, V2B.slot0 + 3, 3), V2F, UF, S2R, RHS2, PL2,
             prec=False, post_s=_norm_chain, rhs_base=RHS1)

    if plan_next is not None:
        plan_next()

    # ---- outputs into the bf16 planar block, deplanarized on ACT ----
    QR, RINV = norm_regs["QR"], norm_regs["RINV"]
    v.tensor_tensor(OPL.ap(3, 4), QR.ap(0, 4), RINV.bcast(0, 4), MULT)
    scrf.free("RINV")
    scrb.free("QR", "S1R", "S2R")

    TMP3 = scrb.alloc("TMP3", 3)
    v.tensor_tensor(TMP3.ap(0, 3), PL1.ap(0, 3), PL2.ap(0, 3), ADD)
    v.scalar_tensor_tensor(OPL.ap(0, 3), TMP3.ap(0, 3), DT / 2,
                           xpos.ap(0, 3), MULT, ADD)
    scrb.free("TMP3", "PL1", "PL2")

    # RHS2 already holds RHS1 + RHS2 (rhs_base fold in dot 2).
    # pos+quat deplanarize on ACT runs early (their planar values are
    # ready before the RHS finishes); the v components are written
    # strided by the DVE so the final out-DMA waits only on the DVE.
    if split_tail:
        qw = W // 4
        pieces = tuple((i * qw, qw) for i in range(4))
    else:
        pieces = ((0, W),)
    for w0, wn in pieces:
        a.activation(oint.ap(0, 7, w0=w0, wn=wn),
                     OPL.ap(0, 7, w0=w0, wn=wn), AF.Copy)
    for w0, wn in pieces:
        for i0, n, mval in _val_runs(mv):
            v.scalar_tensor_tensor(oint.ap(7 + i0, n, w0=w0, wn=wn),
                                   RHS2.ap(i0, n, w0=w0, wn=wn),
                                   DT / 2 * mval,
                                   vF1.ap(i0, n, w0=w0, wn=wn), MULT, ADD)
        if split_tail:
            tail_dmas.append((w0, wn))
    scrf.free("RHS1", "RHS2", "V2F")
    scrb.free("Q2", "V2B")
    scrc.free("OPL")


def emit_dot(nc, pp, scrf, scrb, scrc, q, vl, va, Vrhs, UF, S, RHS, PL,
             prec=False, qf=None, post_s=None, rhs_base=None):
    """One f() evaluation.
    q/vl/va: bf16 planar Regions; Vrhs: fp32 planar v for the RHS path;
    UF: bf16 planar u. With prec=True (dot 1, whose RHS error is amplified
    ~0.2x-abs into the output), the monomials and cross products run fp32
    from the fp32 quat `qf`; with prec=False they run bf16.
    Outputs: S[4] bf16 (2*pDot_ang, S[0] sign-flipped), RHS[6] fp32,
    PL[3] bf16 (pDot_lin).
    """
    v = nc.vector
    a = nc.scalar
    m1, ld, qd, c1, c2 = pp["m1"], pp["ld"], pp["qd"], pp["c1"], pp["c2"]
    scrx = scrc                        # cross products (bf16 both dots)

    # ---- ACT feeder ops first so they overlap the DVE product block ----
    # AB_i = |qd_i * v_i| (free scale on ACT); the DVE folds the linear
    # damping via T6_i = (AB_i + ld_ii) * v_i, then RHS = T6 + u.
    AB = scrf.alloc("AB", 6)
    for i in range(6):
        a.activation(AB.ap(i), Vrhs.ap(i), AF.Abs, scale=float(qd[i]))

    # S rows (2*pDot_ang, S[0] = qv.va sign-flipped)
    TP = scrb.alloc("TP", 13)
    AS = scrb.alloc("AS", 4)
    v.tensor_tensor(TP.ap(0, 3), q.ap(0, 3), va.ap(0, 3), MULT)
    v.tensor_tensor(TP.ap(5, 3, -1), q.ap(1, 3), va.ap(2, 3, -1), MULT)
    v.tensor_tensor(TP.ap(6, 2), q.ap(2, 2), va.ap(0, 2), MULT)
    v.tensor_tensor(TP.ap(8), q.ap(0), va.ap(2), MULT)
    v.tensor_tensor(TP.ap(9, 2), q.ap(0, 2, 3), va.ap(1, 2), MULT)
    v.tensor_tensor(TP.ap(12), q.ap(1), va.ap(0), MULT)
    v.tensor_tensor(AS.ap(0), TP.ap(0), TP.ap(1), ADD)
    v.tensor_tensor(AS.ap(1), TP.ap(3), TP.ap(5), ADD)
    v.tensor_tensor(AS.ap(2, 2), TP.ap(6, 2, 3), TP.ap(7, 2, 3), ADD)
    v.tensor_tensor(S.ap(0), AS.ap(0), TP.ap(2), ADD)
    v.tensor_tensor(S.ap(1, 3), AS.ap(1, 3), TP.ap(4, 3, 4), SUB)
    scrb.free("TP", "AS")
    if post_s is not None:
        post_s()

    # quad monomials: squares on ACT, cross products on DVE
    QD4 = scrb.alloc("QD4", 4)   # [yy, zz, xx, yy]
    P1 = scrb.alloc("P1", 3)     # [xy, xz, yz]
    P2 = scrb.alloc("P2", 3)     # [zw, yw, xw]
    a.activation(QD4.ap(0, 2), q.ap(1, 2), AF.Square)              # yy, zz
    a.activation(QD4.ap(2, 2), q.ap(0, 2), AF.Square)              # xx, yy
    v.tensor_tensor(P1.ap(0, 2), q.bcast(0, 2), q.ap(1, 2), MULT)  # xy, xz
    v.tensor_tensor(P1.ap(2), q.ap(1), q.ap(2), MULT)              # yz
    v.tensor_tensor(P2.ap(0, 2), q.bcast(3, 2), q.ap(2, 2, -1), MULT)
    v.tensor_tensor(P2.ap(2), q.ap(0), q.ap(3), MULT)              # xw

    QO = scrb.alloc("QO", 6)     # [Qo10, Qo02, Qo21, Qo01, Qo20, Qo12]
    QDG = scrb.alloc("QDG", 3)   # (yy+zz, zz+xx, xx+yy)
    v.tensor_tensor(QO.ap(0, 3), P1.ap(0, 3), P2.ap(0, 3), ADD)
    v.tensor_tensor(QO.ap(3, 3), P1.ap(0, 3), P2.ap(0, 3), SUB)
    v.tensor_tensor(QDG.ap(0, 3), QD4.ap(0, 3), QD4.ap(1, 3), ADD)
    scrb.free("QD4", "P1", "P2")
    if prec:
        # dedicated fp32 path for the three error-amplified g-term inputs:
        # GF = [Qo20, Qo21, xx+yy], computed from the fp32 quat qf.
        GSQ = scrf.alloc("GSQ", 2)
        MN = scrf.alloc("MN", 4)     # [xz, yz, yw, xw]
        GF = scrf.alloc("GF", 3)
        a.activation(GSQ.ap(0, 2), qf.ap(0, 2), AF.Square)         # xx, yy
        v.tensor_tensor(MN.ap(0, 2), qf.ap(0, 2), qf.bcast(2, 2), MULT)
        v.tensor_tensor(MN.ap(2, 2), qf.ap(1, 2, -1), qf.bcast(3, 2), MULT)
        v.tensor_tensor(GF.ap(0), MN.ap(0), MN.ap(2), SUB)         # Qo20
        v.tensor_tensor(GF.ap(1), MN.ap(1), MN.ap(3), ADD)         # Qo21
        v.tensor_tensor(GF.ap(2), GSQ.ap(0), GSQ.ap(1), ADD)       # xx+yy
        scrf.free("GSQ", "MN")

    # pDot_lin = vl + 2*(Q @ vl)
    QOr, QDGr = QO, QDG
    RD = scrb.alloc("RD", 3)
    RO = scrb.alloc("RO", 6)     # [R01, R02, R10, R12, R20, R21]
    T1 = scrb.alloc("T1", 3)
    v.tensor_tensor(RD.ap(0, 3), QDGr.ap(0, 3), vl.ap(0, 3), MULT)
    v.tensor_tensor(RO.ap(2, 2, -1), QOr.ap(0, 2), vl.ap(0, 2, 2), MULT)
    v.tensor_tensor(RO.ap(0, 2, 4), QOr.ap(3, 2), vl.ap(1, 2, -1), MULT)
    v.tensor_tensor(RO.ap(5), QOr.ap(2), vl.ap(1), MULT)           # R21
    v.tensor_tensor(RO.ap(3), QOr.ap(5), vl.ap(2), MULT)           # R12
    v.tensor_tensor(T1.ap(0, 3), RO.ap(0, 3, 2), RO.ap(1, 3, 2), ADD)
    v.tensor_tensor(T1.ap(0, 3), T1.ap(0, 3), RD.ap(0, 3), SUB)
    v.scalar_tensor_tensor(PL.ap(0, 3), T1.ap(0, 3), 2.0, vl.ap(0, 3),
                           MULT, ADD)
    scrb.free("RD", "RO", "T1")

    # coriolis cross product va x vl (bf16; error impact measured small)
    PA = scrx.alloc("PA", 3)
    PB = scrx.alloc("PB", 3)
    CR = scrx.alloc("CR", 3)
    v.tensor_tensor(PA.ap(0, 2), va.ap(1, 2), vl.ap(2, 2, -2), MULT)
    v.tensor_tensor(PA.ap(2), va.ap(0), vl.ap(1), MULT)
    v.tensor_tensor(PB.ap(1, 2), va.ap(0, 2), vl.ap(2, 2, -2), MULT)
    v.tensor_tensor(PB.ap(0), va.ap(2), vl.ap(1), MULT)
    v.tensor_tensor(CR.ap(0, 3), PA.ap(0, 3), PB.ap(0, 3), SUB)
    scrx.free("PA", "PB")

    # T6_i = (|qd_i v_i| -sgn(qd_i)*ld_ii) * v_i per equal-(ld,sgn) run;
    # RHS below applies u -sgn(qd_i)*T6, recovering (ld + qd|v|)v + u.
    T6 = scrf.alloc("T6", 6)
    ldd = np.diag(ld)
    sgn = [-1.0 if qd[i] <= 0 else 1.0 for i in range(6)]
    for i0, n, sv in _val_runs([sgn[i] * ldd[i] for i in range(6)]):
        v.scalar_tensor_tensor(T6.ap(i0, n), AB.ap(i0, n), float(sv),
                               Vrhs.ap(i0, n), ADD, MULT)

    # rhs = (ld + qd|v|) v + u - m1*(va x vl) + g-terms  (fp32 accumulation)
    for i0, n, sv in _val_runs(sgn):
        v.tensor_tensor(RHS.ap(i0, n), UF.ap(i0, n), T6.ap(i0, n),
                        SUB if sv < 0 else ADD)
    if rhs_base is not None:
        v.tensor_tensor(RHS.ap(0, 6), RHS.ap(0, 6), rhs_base.ap(0, 6), ADD)
    for i in range(6):
        for j in range(6):
            if i != j and ld[i, j] != 0.0:
                v.scalar_tensor_tensor(RHS.ap(i), Vrhs.ap(j), float(ld[i, j]),
                                       RHS.ap(i), MULT, ADD)
    scrf.free("AB", "T6")
    v.scalar_tensor_tensor(RHS.ap(0, 3), CR.ap(0, 3), -m1, RHS.ap(0, 3),
                           MULT, ADD)
    scrx.free("CR")

    # restoring: rot2 = (2*Qo20, 2*Qo21, 1-2*Qd2); rhs += [c1*rot2; c2 x rot2]
    TMPG = scrf.alloc("TMPG", 1)
    if prec:
        ap2021 = GF.ap(0, 2)           # (Qo20, Qo21)
        s21, s20, sd2 = (GF, 1), (GF, 0), (GF, 2)
    else:
        ap2021 = QO.ap(4, 2, -2)
        s21, s20, sd2 = (QO, 2), (QO, 4), (QDG, 2)
    v.scalar_tensor_tensor(RHS.ap(0, 2), ap2021, 2.0 * c1,
                           RHS.ap(0, 2), MULT, ADD)
    v.tensor_scalar(TMPG.ap(0), sd2[0].ap(sd2[1]), -2.0 * c1, c1, MULT, ADD)
    v.tensor_tensor(RHS.ap(2), RHS.ap(2), TMPG.ap(0), ADD)

    c2x, c2y, c2z = (float(c2[0]), float(c2[1]), float(c2[2]))
    gterms = [
        (3, [s21 + (-2.0 * c2z,), sd2 + (-2.0 * c2y,)], c2y),
        (4, [s20 + (2.0 * c2z,), sd2 + (2.0 * c2x,)], -c2x),
        (5, [s21 + (2.0 * c2x,), s20 + (-2.0 * c2y,)], 0.0),
    ]
    for row, terms, const in gterms:
        terms = [(reg, s, co) for (reg, s, co) in terms if co != 0.0]
        if const != 0.0:
            if terms:
                reg, s, co = terms.pop(0)
                v.tensor_scalar(TMPG.ap(0), reg.ap(s), co, const, MULT, ADD)
                v.tensor_tensor(RHS.ap(row), RHS.ap(row), TMPG.ap(0), ADD)
            else:
                v.tensor_scalar(RHS.ap(row), RHS.ap(row), const, None, ADD)
        for reg, s, co in terms:
            v.scalar_tensor_tensor(RHS.ap(row), reg.ap(s), co, RHS.ap(row),
                                   MULT, ADD)
    scrf.free("TMPG")
    scrb.free("QO", "QDG")
    if prec:
        scrf.free("GF")


_CACHE = {}


def kernel(**inputs):
    from concourse.bass_utils import run_bass_kernel_spmd

    x = np.ascontiguousarray(np.asarray(inputs["x"], np.float32))
    u = np.ascontiguousarray(np.asarray(inputs["u"], np.float32))
    K = x.shape[0]
    assert K % N_CORES == 0
    K_core = K // N_CORES
    W = 256
    assert K_core % (P * W) == 0

    pp = _extract_params(inputs)
    pp_key = (K_core, W, pp["m1"], pp["c1"], tuple(pp["minv"]),
              tuple(pp["qd"]), tuple(pp["c2"]), pp["ld"].tobytes())
    if pp_key not in _CACHE:
        _CACHE[pp_key] = build_program(pp, K_core, W)
    nc = _CACHE[pp_key]

    in_maps = []
    for k in range(N_CORES):
        sl = slice(k * K_core, (k + 1) * K_core)
        in_maps.append({"x": x[sl], "u": u[sl]})

    kwargs = dict(trace=True) if TRACE else {}
    res = run_bass_kernel_spmd(nc, in_maps, core_ids=list(range(N_CORES)),
                               **kwargs)
    LAST_RUN_INFO.clear()
    LAST_RUN_INFO.update(dict(
        exec_time_ns=res.exec_time_ns,
        mean_exec_time_ns=res.mean_exec_time_ns,
        profile_json=res.profile_json,
    ))
    out = np.empty((K, 13), np.float32)
    for k in range(N_CORES):
        out[k * K_core:(k + 1) * K_core] = res.results[k]["o"]
    return out
